# revision 10
# baseline (speedup 1.0000x reference)
"""Trainium2 Bass kernel for nn_MoELayer (top-2-of-8 MoE, T=1024 B=4 D=1024 H=2048).

Strategy: pure data-parallel over tokens. Each of the 8 NeuronCores takes
512 tokens and computes layernorm -> router -> top-2 softmax weights ->
dense 8-expert MLP (fp32r matmuls) -> weighted combine, locally.
No collectives. Host only reshapes/slices inputs and concatenates outputs.
"""

import sys

sys.path.insert(0, "/opt/trn_rl_repo")

import numpy as np

import concourse.bacc as bacc
import concourse.mybir as mybir
import concourse.tile as tile
from concourse.bass_utils import run_bass_kernel_spmd
from concourse.masks import make_identity

T, B, D = 1024, 4, 1024
E, TOPK = 8, 2
H = 2048
O = D
NCORES = 8
NTOK = T * B          # 4096
TPC = NTOK // NCORES  # 512 tokens per core
P = 128
TOK_TILES = TPC // P  # 4
KD = D // P           # 8  k-chunks for D contraction
KH = H // P           # 16 k-chunks for H contraction
MH = H // P           # 16 m-tiles of H
F32 = mybir.dt.float32
FR = mybir.dt.float32r
AX = mybir.AxisListType.X
GELU = mybir.ActivationFunctionType.Gelu
EXP = mybir.ActivationFunctionType.Exp
RSQRT = mybir.ActivationFunctionType.Rsqrt


def _build_kernel():
    nc = bacc.Bacc("TRN2", target_bir_lowering=False, debug=False,
                   num_devices=NCORES)

    x_d = nc.dram_tensor("x", [TPC, D], F32, kind="ExternalInput").ap()
    sm_d = nc.dram_tensor("seqmask", [TPC, 1], F32, kind="ExternalInput").ap()
    lns_d = nc.dram_tensor("ln_scale", [1, D], F32, kind="ExternalInput").ap()
    lnb_d = nc.dram_tensor("ln_bias", [1, D], F32, kind="ExternalInput").ap()
    rwt_d = nc.dram_tensor("rwT", [D, E], FR, kind="ExternalInput").ap()
    w1_d = nc.dram_tensor("w1", [E, D, H], FR, kind="ExternalInput").ap()
    b1_d = nc.dram_tensor("b1", [E, H], F32, kind="ExternalInput").ap()
    w2_d = nc.dram_tensor("w2", [E, H, O], FR, kind="ExternalInput").ap()
    b2_d = nc.dram_tensor("b2", [E, O], F32, kind="ExternalInput").ap()
    out_d = nc.dram_tensor("out", [TPC, O], F32, kind="ExternalOutput").ap()
    gate_d = nc.dram_tensor("gate", [TPC, E], F32, kind="ExternalOutput").ap()

    with tile.TileContext(nc) as tc:
        with (
            tc.tile_pool(name="const", bufs=1) as const,
            tc.tile_pool(name="route", bufs=1) as route,
            tc.tile_pool(name="xnt", bufs=1) as xnt_p,
            tc.tile_pool(name="ht", bufs=1) as ht_p,
            tc.tile_pool(name="accp", bufs=1) as acc_p,
            tc.tile_pool(name="w1p", bufs=1) as w1_p,
            tc.tile_pool(name="w2p", bufs=3) as w2_p,
            tc.tile_pool(name="ln", bufs=2) as ln_p,
            tc.tile_pool(name="ev", bufs=4) as ev_p,
            tc.tile_pool(name="b2b", bufs=2) as b2_p,
            tc.tile_pool(name="b2r", bufs=1) as b2r_p,
            tc.tile_pool(name="psA", bufs=2, space="PSUM") as psA,
            tc.tile_pool(name="ps1p", bufs=2, space="PSUM") as ps1p,
            tc.tile_pool(name="ps2p", bufs=1, space="PSUM") as ps2p,
        ):
            ident = const.tile([P, P], F32)
            make_identity(nc, ident[:])
            eps_t = const.tile([P, 1], F32)
            nc.vector.memset(eps_t[:], 1e-5)

            # ln scale/bias, one column per D-chunk (per-partition in xn_T)
            lns_pc = const.tile([P, KD], F32)
            nc.sync.dma_start(
                out=lns_pc[:], in_=lns_d.rearrange("o (k p) -> p (o k)", p=P))
            lnb_pc = const.tile([P, KD], F32)
            nc.sync.dma_start(
                out=lnb_pc[:], in_=lnb_d.rearrange("o (k p) -> p (o k)", p=P))

            rwt_sb = const.tile([P, KD, E], FR)
            nc.sync.dma_start(
                out=rwt_sb[:], in_=rwt_d.rearrange("(k p) e -> p k e", p=P))
            b1t_sb = const.tile([P, E, H // P], F32)
            nc.sync.dma_start(
                out=b1t_sb[:], in_=b1_d.rearrange("e (m p) -> p e m", p=P))
            sm_sb = const.tile([P, TOK_TILES, 1], F32)
            nc.sync.dma_start(
                out=sm_sb[:], in_=sm_d.rearrange("(t p) o -> p t o", p=P))

            xn_t = xnt_p.tile([P, KD, TPC], FR)     # xn^T  [D, tok]
            h_t = ht_p.tile([P, KH, TPC], FR)       # h^T   [H, tok]
            acc = acc_p.tile([P, TOK_TILES, O], F32)  # out   [tok, O]
            w_sb = route.tile([P, TOK_TILES, E], F32)  # combine weights

            # ---- LayerNorm + transpose + router + top2 weights ----
            for i in range(TOK_TILES):
                xt = ln_p.tile([P, D], F32, name="xt")
                nc.sync.dma_start(out=xt[:], in_=x_d[i * P:(i + 1) * P, :])
                s = ln_p.tile([P, 1], F32, name="s")
                nc.vector.reduce_sum(s[:], xt[:], axis=AX)
                mu = ln_p.tile([P, 1], F32, name="mu")
                nc.scalar.mul(mu[:], s[:], 1.0 / D)
                xc = xt
                nc.vector.tensor_sub(xc[:], xt[:], mu[:].to_broadcast([P, D]))
                sq = ln_p.tile([P, D], F32, name="sq")
                nc.vector.tensor_mul(sq[:], xc[:], xc[:])
                ssq = ln_p.tile([P, 1], F32, name="ssq")
                nc.vector.reduce_sum(ssq[:], sq[:], axis=AX)
                vtmp = ln_p.tile([P, 1], F32, name="vtmp")
                nc.scalar.mul(vtmp[:], ssq[:], 1.0 / D)
                std = ln_p.tile([P, 1], F32, name="std")
                nc.scalar.activation(std[:], vtmp[:],
                                     mybir.ActivationFunctionType.Sqrt,
                                     bias=eps_t[:])
                rstd = ln_p.tile([P, 1], F32, name="rstd")
                nc.vector.reciprocal(rstd[:], std[:])
                xn = sq
                nc.vector.tensor_scalar_mul(xn[:], xc[:], rstd[:])

                # transpose 128x128 blocks into xn_t
                for k in range(KD):
                    pt = psA.tile([P, 512], F32, name="psa")[:, :P]
                    nc.tensor.transpose(pt[:], xn[:, k * P:(k + 1) * P],
                                        ident[:])
                    nc.vector.tensor_scalar(
                        out=xn_t[:, k, i * P:(i + 1) * P], in0=pt[:],
                        scalar1=lns_pc[:, k:k + 1], scalar2=lnb_pc[:, k:k + 1],
                        op0=mybir.AluOpType.mult, op1=mybir.AluOpType.add)

                # router logits for this token tile
                pg = psA.tile([P, 512], F32, name="psa")[:, :E]
                for k in range(KD):
                    nc.tensor.matmul(
                        pg[:], lhsT=xn_t[:, k, i * P:(i + 1) * P],
                        rhs=rwt_sb[:, k, :],
                        start=(k == 0), stop=(k == KD - 1))
                gt = ln_p.tile([P, E], F32, name="gt")
                nc.vector.tensor_copy(gt[:], pg[:])
                nc.sync.dma_start(out=gate_d[i * P:(i + 1) * P, :], in_=gt[:])

                # top-2 softmax -> dense combine weights, masked
                mx = ln_p.tile([P, 8], F32, name="mx")
                nc.vector.max(out=mx[:], in_=gt[:])
                df = ln_p.tile([P, E], F32, name="df")
                nc.vector.tensor_sub(df[:], gt[:],
                                     mx[:, 0:1].to_broadcast([P, E]))
                ex = ln_p.tile([P, E], F32, name="ex")
                nc.scalar.activation(ex[:], df[:], EXP)
                mge = ln_p.tile([P, E], F32, name="mge")
                nc.vector.tensor_tensor(out=mge[:], in0=gt[:],
                                        in1=mx[:, 1:2].to_broadcast([P, E]),
                                        op=mybir.AluOpType.is_ge)
                num = ln_p.tile([P, E], F32, name="num")
                nc.vector.tensor_mul(num[:], ex[:], mge[:])
                den = ln_p.tile([P, 1], F32, name="den")
                nc.vector.reduce_sum(den[:], num[:], axis=AX)
                rden = ln_p.tile([P, 1], F32, name="rden")
                nc.vector.reciprocal(rden[:], den[:])
                nc.vector.tensor_mul(rden[:], rden[:], sm_sb[:, i, :])
                nc.vector.tensor_scalar_mul(w_sb[:, i, :], num[:], rden[:])

            # ---- dense expert MLPs ----
            for e in range(E):
                w1_sb = w1_p.tile([P, KD, H], FR, name="w1sb")
                nc.sync.dma_start(
                    out=w1_sb[:],
                    in_=w1_d[e].rearrange("(k p) h -> p k h", p=P))

                # layer 1: h^T[H, tok] = gelu(w1^T xn + b1)
                for m in range(MH):
                    ps = ps1p.tile([P, 512], F32, name="ps1")
                    for k in range(KD):
                        nc.tensor.matmul(
                            ps[:], lhsT=w1_sb[:, k, m * P:(m + 1) * P],
                            rhs=xn_t[:, k, :],
                            start=(k == 0), stop=(k == KD - 1))
                    nc.scalar.activation(h_t[:, m, :], ps[:], GELU,
                                         bias=b1t_sb[:, e, m:m + 1])

                # layer 2: out[tok, O] += w[tok,e] * gelu(h w2 + b2)
                b2row = b2r_p.tile([1, O], F32, name="b2row")
                nc.sync.dma_start(out=b2row[:], in_=b2_d[e][None, :])
                b2bc = b2_p.tile([P, O], F32, name="b2bc")
                nc.gpsimd.partition_broadcast(b2bc[:], b2row[:])
                for o in range(2):
                    pts = [ps2p.tile([P, 512], F32, name=f"ps2_{t}")
                           for t in range(TOK_TILES)]
                    for k in range(KH):
                        w2k = w2_p.tile([P, 512], FR, name="w2k")
                        nc.sync.dma_start(
                            out=w2k[:],
                            in_=w2_d[e, k * P:(k + 1) * P,
                                     o * 512:(o + 1) * 512])
                        for t in range(TOK_TILES):
                            nc.tensor.matmul(
                                pts[t][:],
                                lhsT=h_t[:, k, t * P:(t + 1) * P],
                                rhs=w2k[:],
                                start=(k == 0), stop=(k == KH - 1))
                    for t in range(TOK_TILES):
                        y = ev_p.tile([P, 512], F32, name="y")
                        nc.vector.tensor_add(y[:], pts[t][:],
                                             b2bc[:, o * 512:(o + 1) * 512])
                        nc.scalar.activation(y[:], y[:], GELU)
                        osl = acc[:, t, o * 512:(o + 1) * 512]
                        if e == 0:
                            nc.vector.tensor_scalar_mul(
                                osl, y[:], w_sb[:, t, e:e + 1])
                        else:
                            nc.vector.tensor_scalar_mul(
                                y[:], y[:], w_sb[:, t, e:e + 1])
                            nc.vector.tensor_add(osl, osl, y[:])

            for t in range(TOK_TILES):
                nc.sync.dma_start(out=out_d[t * P:(t + 1) * P, :],
                                  in_=acc[:, t, :])

    nc.compile()
    return nc


_NC_CACHE = None


def _get_nc():
    global _NC_CACHE
    if _NC_CACHE is None:
        _NC_CACHE = _build_kernel()
    return _NC_CACHE


def kernel(x, sequence_mask, ln_scale, ln_bias, router_w, w1, b1, w2, b2):
    x = np.asarray(x, np.float32).reshape(NTOK, D)
    sm = np.asarray(sequence_mask, np.float32).reshape(NTOK, 1)
    lns = np.asarray(ln_scale, np.float32).reshape(1, D)
    lnb = np.asarray(ln_bias, np.float32).reshape(1, D)
    rwt = np.ascontiguousarray(np.asarray(router_w, np.float32).T)
    w1 = np.ascontiguousarray(np.asarray(w1, np.float32))
    b1 = np.ascontiguousarray(np.asarray(b1, np.float32))
    w2 = np.ascontiguousarray(np.asarray(w2, np.float32))
    b2 = np.ascontiguousarray(np.asarray(b2, np.float32))

    nc = _get_nc()
    in_maps = []
    for c in range(NCORES):
        sl = slice(c * TPC, (c + 1) * TPC)
        in_maps.append({
            "x": np.ascontiguousarray(x[sl]),
            "seqmask": np.ascontiguousarray(sm[sl]),
            "ln_scale": lns, "ln_bias": lnb, "rwT": rwt,
            "w1": w1, "b1": b1, "w2": w2, "b2": b2,
        })
    res = run_bass_kernel_spmd(nc, in_maps, core_ids=list(range(NCORES)))
    out = np.concatenate([res.results[c]["out"] for c in range(NCORES)], 0)
    gate = np.concatenate([res.results[c]["gate"] for c in range(NCORES)], 0)
    return out.reshape(T, B, O), gate


# revision 11
# speedup vs baseline: 1.0609x; 1.0609x over previous
"""Trainium2 Bass kernel for nn_MoELayer (top-2-of-8 MoE, T=1024 B=4 D=1024 H=2048).

Strategy: pure data-parallel over tokens. Each of the 8 NeuronCores takes
512 tokens and computes layernorm -> router -> top-2 softmax weights ->
dense 8-expert MLP (fp32r matmuls) -> weighted combine, locally.
No collectives. Host only reshapes/slices inputs and concatenates outputs.
"""

import sys

sys.path.insert(0, "/opt/trn_rl_repo")

import numpy as np

import concourse.bacc as bacc
import concourse.mybir as mybir
import concourse.tile as tile
from concourse.bass_utils import run_bass_kernel_spmd
from concourse.masks import make_identity

T, B, D = 1024, 4, 1024
E, TOPK = 8, 2
H = 2048
O = D
NCORES = 8
NTOK = T * B          # 4096
TPC = NTOK // NCORES  # 512 tokens per core
P = 128
TOK_TILES = TPC // P  # 4
KD = D // P           # 8  k-chunks for D contraction
KH = H // P           # 16 k-chunks for H contraction
MH = H // P           # 16 m-tiles of H
F32 = mybir.dt.float32
FR = mybir.dt.float32r
F16 = mybir.dt.float16
AX = mybir.AxisListType.X
GELU = mybir.ActivationFunctionType.Gelu
EXP = mybir.ActivationFunctionType.Exp
RSQRT = mybir.ActivationFunctionType.Rsqrt


def _build_kernel():
    nc = bacc.Bacc("TRN2", target_bir_lowering=False, debug=False,
                   num_devices=NCORES)

    x_d = nc.dram_tensor("x", [TPC, D], F32, kind="ExternalInput").ap()
    sm_d = nc.dram_tensor("seqmask", [TPC, 1], F32, kind="ExternalInput").ap()
    lns_d = nc.dram_tensor("ln_scale", [1, D], F32, kind="ExternalInput").ap()
    lnb_d = nc.dram_tensor("ln_bias", [1, D], F32, kind="ExternalInput").ap()
    rwt_d = nc.dram_tensor("rwT", [D, E], FR, kind="ExternalInput").ap()
    w1_d = nc.dram_tensor("w1", [E, D, H], F32, kind="ExternalInput").ap()
    b1_d = nc.dram_tensor("b1", [E, H], F32, kind="ExternalInput").ap()
    w2_d = nc.dram_tensor("w2", [E, H, O], F32, kind="ExternalInput").ap()
    b2_d = nc.dram_tensor("b2", [E, O], F32, kind="ExternalInput").ap()
    out_d = nc.dram_tensor("out", [TPC, O], F32, kind="ExternalOutput").ap()
    gate_d = nc.dram_tensor("gate", [TPC, E], F32, kind="ExternalOutput").ap()

    with tile.TileContext(nc) as tc:
        with (
            tc.tile_pool(name="const", bufs=1) as const,
            tc.tile_pool(name="route", bufs=1) as route,
            tc.tile_pool(name="xnt", bufs=1) as xnt_p,
            tc.tile_pool(name="ht", bufs=1) as ht_p,
            tc.tile_pool(name="accp", bufs=1) as acc_p,
            tc.tile_pool(name="w1p", bufs=1) as w1_p,
            tc.tile_pool(name="wraw", bufs=2) as wraw_p,
            tc.tile_pool(name="w2p", bufs=3) as w2_p,
            tc.tile_pool(name="ln", bufs=2) as ln_p,
            tc.tile_pool(name="ev", bufs=4) as ev_p,
            tc.tile_pool(name="b2b", bufs=2) as b2_p,
            tc.tile_pool(name="b2r", bufs=1) as b2r_p,
            tc.tile_pool(name="psA", bufs=2, space="PSUM") as psA,
            tc.tile_pool(name="ps1p", bufs=2, space="PSUM") as ps1p,
            tc.tile_pool(name="ps2p", bufs=1, space="PSUM") as ps2p,
        ):
            ident = const.tile([P, P], F32)
            make_identity(nc, ident[:])
            eps_t = const.tile([P, 1], F32)
            nc.vector.memset(eps_t[:], 1e-5)

            # ln scale/bias, one column per D-chunk (per-partition in xn_T)
            lns_pc = const.tile([P, KD], F32)
            nc.sync.dma_start(
                out=lns_pc[:], in_=lns_d.rearrange("o (k p) -> p (o k)", p=P))
            lnb_pc = const.tile([P, KD], F32)
            nc.sync.dma_start(
                out=lnb_pc[:], in_=lnb_d.rearrange("o (k p) -> p (o k)", p=P))

            rwt_sb = const.tile([P, KD, E], FR)
            nc.sync.dma_start(
                out=rwt_sb[:], in_=rwt_d.rearrange("(k p) e -> p k e", p=P))
            b1t_sb = const.tile([P, E, H // P], F32)
            nc.sync.dma_start(
                out=b1t_sb[:], in_=b1_d.rearrange("e (m p) -> p e m", p=P))
            sm_sb = const.tile([P, TOK_TILES, 1], F32)
            nc.sync.dma_start(
                out=sm_sb[:], in_=sm_d.rearrange("(t p) o -> p t o", p=P))

            xn_t = xnt_p.tile([P, KD, TPC], FR)     # xn^T  [D, tok] (router)
            xn_t16 = xnt_p.tile([P, KD, TPC], F16)  # xn^T fp16 (experts)
            h_t = ht_p.tile([P, KH, TPC], F16)      # h^T   [H, tok]
            acc = acc_p.tile([P, TOK_TILES, O], F32)  # out   [tok, O]
            w_sb = route.tile([P, TOK_TILES, E], F32)  # combine weights

            # ---- LayerNorm + transpose + router + top2 weights ----
            for i in range(TOK_TILES):
                xt = ln_p.tile([P, D], F32, name="xt")
                nc.sync.dma_start(out=xt[:], in_=x_d[i * P:(i + 1) * P, :])
                s = ln_p.tile([P, 1], F32, name="s")
                nc.vector.reduce_sum(s[:], xt[:], axis=AX)
                mu = ln_p.tile([P, 1], F32, name="mu")
                nc.scalar.mul(mu[:], s[:], 1.0 / D)
                xc = xt
                nc.vector.tensor_sub(xc[:], xt[:], mu[:].to_broadcast([P, D]))
                sq = ln_p.tile([P, D], F32, name="sq")
                nc.vector.tensor_mul(sq[:], xc[:], xc[:])
                ssq = ln_p.tile([P, 1], F32, name="ssq")
                nc.vector.reduce_sum(ssq[:], sq[:], axis=AX)
                vtmp = ln_p.tile([P, 1], F32, name="vtmp")
                nc.scalar.mul(vtmp[:], ssq[:], 1.0 / D)
                std = ln_p.tile([P, 1], F32, name="std")
                nc.scalar.activation(std[:], vtmp[:],
                                     mybir.ActivationFunctionType.Sqrt,
                                     bias=eps_t[:])
                rstd = ln_p.tile([P, 1], F32, name="rstd")
                nc.vector.reciprocal(rstd[:], std[:])
                xn = sq
                nc.vector.tensor_scalar_mul(xn[:], xc[:], rstd[:])

                # transpose 128x128 blocks into xn_t
                for k in range(KD):
                    pt = psA.tile([P, 512], F32, name="psa")[:, :P]
                    nc.tensor.transpose(pt[:], xn[:, k * P:(k + 1) * P],
                                        ident[:])
                    nc.vector.tensor_scalar(
                        out=xn_t[:, k, i * P:(i + 1) * P], in0=pt[:],
                        scalar1=lns_pc[:, k:k + 1], scalar2=lnb_pc[:, k:k + 1],
                        op0=mybir.AluOpType.mult, op1=mybir.AluOpType.add)
                    nc.vector.tensor_copy(
                        xn_t16[:, k, i * P:(i + 1) * P],
                        xn_t[:, k, i * P:(i + 1) * P])

                # router logits for this token tile
                pg = psA.tile([P, 512], F32, name="psa")[:, :E]
                for k in range(KD):
                    nc.tensor.matmul(
                        pg[:], lhsT=xn_t[:, k, i * P:(i + 1) * P],
                        rhs=rwt_sb[:, k, :],
                        start=(k == 0), stop=(k == KD - 1))
                gt = ln_p.tile([P, E], F32, name="gt")
                nc.vector.tensor_copy(gt[:], pg[:])
                nc.sync.dma_start(out=gate_d[i * P:(i + 1) * P, :], in_=gt[:])

                # top-2 softmax -> dense combine weights, masked
                mx = ln_p.tile([P, 8], F32, name="mx")
                nc.vector.max(out=mx[:], in_=gt[:])
                df = ln_p.tile([P, E], F32, name="df")
                nc.vector.tensor_sub(df[:], gt[:],
                                     mx[:, 0:1].to_broadcast([P, E]))
                ex = ln_p.tile([P, E], F32, name="ex")
                nc.scalar.activation(ex[:], df[:], EXP)
                mge = ln_p.tile([P, E], F32, name="mge")
                nc.vector.tensor_tensor(out=mge[:], in0=gt[:],
                                        in1=mx[:, 1:2].to_broadcast([P, E]),
                                        op=mybir.AluOpType.is_ge)
                num = ln_p.tile([P, E], F32, name="num")
                nc.vector.tensor_mul(num[:], ex[:], mge[:])
                den = ln_p.tile([P, 1], F32, name="den")
                nc.vector.reduce_sum(den[:], num[:], axis=AX)
                rden = ln_p.tile([P, 1], F32, name="rden")
                nc.vector.reciprocal(rden[:], den[:])
                nc.vector.tensor_mul(rden[:], rden[:], sm_sb[:, i, :])
                nc.vector.tensor_scalar_mul(w_sb[:, i, :], num[:], rden[:])

            # ---- dense expert MLPs ----
            for e in range(E):
                w1_sb = w1_p.tile([P, KD, H], F16, name="w1sb")
                for k in range(KD):
                    wr = wraw_p.tile([P, H], F32, name="wr")
                    nc.sync.dma_start(
                        out=wr[:], in_=w1_d[e, k * P:(k + 1) * P, :])
                    nc.vector.tensor_copy(w1_sb[:, k, :], wr[:])

                # layer 1: h^T[H, tok] = gelu(w1^T xn + b1)
                for m in range(MH):
                    ps = ps1p.tile([P, 512], F32, name="ps1")
                    for k in range(KD):
                        nc.tensor.matmul(
                            ps[:], lhsT=w1_sb[:, k, m * P:(m + 1) * P],
                            rhs=xn_t16[:, k, :],
                            start=(k == 0), stop=(k == KD - 1))
                    nc.scalar.activation(h_t[:, m, :], ps[:], GELU,
                                         bias=b1t_sb[:, e, m:m + 1])

                # layer 2: out[tok, O] += w[tok,e] * gelu(h w2 + b2)
                b2row = b2r_p.tile([1, O], F32, name="b2row")
                nc.sync.dma_start(out=b2row[:], in_=b2_d[e][None, :])
                b2bc = b2_p.tile([P, O], F32, name="b2bc")
                nc.gpsimd.partition_broadcast(b2bc[:], b2row[:])
                for o in range(2):
                    pts = [ps2p.tile([P, 512], F32, name=f"ps2_{t}")
                           for t in range(TOK_TILES)]
                    for k in range(KH):
                        w2r = wraw_p.tile([P, 512], F32, name="w2r")
                        nc.sync.dma_start(
                            out=w2r[:],
                            in_=w2_d[e, k * P:(k + 1) * P,
                                     o * 512:(o + 1) * 512])
                        w2k = w2_p.tile([P, 512], F16, name="w2k")
                        nc.scalar.activation(
                            w2k[:], w2r[:],
                            mybir.ActivationFunctionType.Copy)
                        for t in range(TOK_TILES):
                            nc.tensor.matmul(
                                pts[t][:],
                                lhsT=h_t[:, k, t * P:(t + 1) * P],
                                rhs=w2k[:],
                                start=(k == 0), stop=(k == KH - 1))
                    for t in range(TOK_TILES):
                        y = ev_p.tile([P, 512], F32, name="y")
                        nc.vector.tensor_add(y[:], pts[t][:],
                                             b2bc[:, o * 512:(o + 1) * 512])
                        nc.scalar.activation(y[:], y[:], GELU)
                        osl = acc[:, t, o * 512:(o + 1) * 512]
                        if e == 0:
                            nc.vector.tensor_scalar_mul(
                                osl, y[:], w_sb[:, t, e:e + 1])
                        else:
                            nc.vector.tensor_scalar_mul(
                                y[:], y[:], w_sb[:, t, e:e + 1])
                            nc.vector.tensor_add(osl, osl, y[:])

            for t in range(TOK_TILES):
                nc.sync.dma_start(out=out_d[t * P:(t + 1) * P, :],
                                  in_=acc[:, t, :])

    nc.compile()
    return nc


_NC_CACHE = None


def _get_nc():
    global _NC_CACHE
    if _NC_CACHE is None:
        _NC_CACHE = _build_kernel()
    return _NC_CACHE


def kernel(x, sequence_mask, ln_scale, ln_bias, router_w, w1, b1, w2, b2):
    x = np.asarray(x, np.float32).reshape(NTOK, D)
    sm = np.asarray(sequence_mask, np.float32).reshape(NTOK, 1)
    lns = np.asarray(ln_scale, np.float32).reshape(1, D)
    lnb = np.asarray(ln_bias, np.float32).reshape(1, D)
    rwt = np.ascontiguousarray(np.asarray(router_w, np.float32).T)
    w1 = np.ascontiguousarray(np.asarray(w1, np.float32))
    b1 = np.ascontiguousarray(np.asarray(b1, np.float32))
    w2 = np.ascontiguousarray(np.asarray(w2, np.float32))
    b2 = np.ascontiguousarray(np.asarray(b2, np.float32))

    nc = _get_nc()
    in_maps = []
    for c in range(NCORES):
        sl = slice(c * TPC, (c + 1) * TPC)
        in_maps.append({
            "x": np.ascontiguousarray(x[sl]),
            "seqmask": np.ascontiguousarray(sm[sl]),
            "ln_scale": lns, "ln_bias": lnb, "rwT": rwt,
            "w1": w1, "b1": b1, "w2": w2, "b2": b2,
        })
    res = run_bass_kernel_spmd(nc, in_maps, core_ids=list(range(NCORES)))
    out = np.concatenate([res.results[c]["out"] for c in range(NCORES)], 0)
    gate = np.concatenate([res.results[c]["gate"] for c in range(NCORES)], 0)
    return out.reshape(T, B, O), gate


# revision 12
# speedup vs baseline: 1.1803x; 1.1126x over previous
"""Trainium2 Bass kernel for nn_MoELayer (top-2-of-8 MoE, T=1024 B=4 D=1024 H=2048).

Strategy: pure data-parallel over tokens. Each of the 8 NeuronCores takes
512 tokens and computes layernorm -> router -> top-2 softmax weights ->
dense 8-expert MLP (fp32r matmuls) -> weighted combine, locally.
No collectives. Host only reshapes/slices inputs and concatenates outputs.
"""

import sys

sys.path.insert(0, "/opt/trn_rl_repo")

import numpy as np

import concourse.bacc as bacc
import concourse.mybir as mybir
import concourse.tile as tile
from concourse.bass_utils import run_bass_kernel_spmd
from concourse.masks import make_identity

T, B, D = 1024, 4, 1024
E, TOPK = 8, 2
H = 2048
O = D
NCORES = 8
NTOK = T * B          # 4096
TPC = NTOK // NCORES  # 512 tokens per core
P = 128
TOK_TILES = TPC // P  # 4
KD = D // P           # 8  k-chunks for D contraction
KH = H // P           # 16 k-chunks for H contraction
MH = H // P           # 16 m-tiles of H
F32 = mybir.dt.float32
FR = mybir.dt.float32r
F16 = mybir.dt.float16
AX = mybir.AxisListType.X
GELU = mybir.ActivationFunctionType.Gelu
EXP = mybir.ActivationFunctionType.Exp
RSQRT = mybir.ActivationFunctionType.Rsqrt


def _build_kernel():
    nc = bacc.Bacc("TRN2", target_bir_lowering=False, debug=False,
                   num_devices=NCORES)

    x_d = nc.dram_tensor("x", [TPC, D], F32, kind="ExternalInput").ap()
    sm_d = nc.dram_tensor("seqmask", [TPC, 1], F32, kind="ExternalInput").ap()
    lns_d = nc.dram_tensor("ln_scale", [1, D], F32, kind="ExternalInput").ap()
    lnb_d = nc.dram_tensor("ln_bias", [1, D], F32, kind="ExternalInput").ap()
    rwt_d = nc.dram_tensor("rwT", [D, E], FR, kind="ExternalInput").ap()
    w1_d = nc.dram_tensor("w1", [E, D, H], F32, kind="ExternalInput").ap()
    b1_d = nc.dram_tensor("b1", [E, H], F32, kind="ExternalInput").ap()
    w2_d = nc.dram_tensor("w2", [E, H, O], F32, kind="ExternalInput").ap()
    b2_d = nc.dram_tensor("b2", [E, O], F32, kind="ExternalInput").ap()
    out_d = nc.dram_tensor("out", [TPC, O], F32, kind="ExternalOutput").ap()
    gate_d = nc.dram_tensor("gate", [TPC, E], F32, kind="ExternalOutput").ap()

    with tile.TileContext(nc) as tc:
        with (
            tc.tile_pool(name="const", bufs=1) as const,
            tc.tile_pool(name="route", bufs=1) as route,
            tc.tile_pool(name="xnt", bufs=1) as xnt_p,
            tc.tile_pool(name="ht", bufs=1) as ht_p,
            tc.tile_pool(name="accp", bufs=1) as acc_p,
            tc.tile_pool(name="w1p", bufs=1) as w1_p,
            tc.tile_pool(name="wraw", bufs=2) as wraw_p,
            tc.tile_pool(name="w2p", bufs=3) as w2_p,
            tc.tile_pool(name="ln", bufs=2) as ln_p,
            tc.tile_pool(name="ev", bufs=4) as ev_p,
            tc.tile_pool(name="b2b", bufs=2) as b2_p,
            tc.tile_pool(name="b2r", bufs=1) as b2r_p,
            tc.tile_pool(name="ps1p", bufs=4, space="PSUM") as ps1p,
            tc.tile_pool(name="ps2p", bufs=1, space="PSUM") as ps2p,
        ):
            ident = const.tile([P, P], F32)
            make_identity(nc, ident[:])
            eps_t = const.tile([P, 1], F32)
            nc.vector.memset(eps_t[:], 1e-5)

            # ln scale/bias, one column per D-chunk (per-partition in xn_T)
            lns_pc = const.tile([P, KD], F32)
            nc.sync.dma_start(
                out=lns_pc[:], in_=lns_d.rearrange("o (k p) -> p (o k)", p=P))
            lnb_pc = const.tile([P, KD], F32)
            nc.sync.dma_start(
                out=lnb_pc[:], in_=lnb_d.rearrange("o (k p) -> p (o k)", p=P))

            rwt_sb = const.tile([P, KD, E], FR)
            nc.sync.dma_start(
                out=rwt_sb[:], in_=rwt_d.rearrange("(k p) e -> p k e", p=P))
            b1t_sb = const.tile([P, E, H // P], F32)
            nc.sync.dma_start(
                out=b1t_sb[:], in_=b1_d.rearrange("e (m p) -> p e m", p=P))
            sm_sb = const.tile([P, TOK_TILES, 1], F32)
            nc.sync.dma_start(
                out=sm_sb[:], in_=sm_d.rearrange("(t p) o -> p t o", p=P))

            xn_t = xnt_p.tile([P, KD, TPC], FR)     # xn^T  [D, tok] (router)
            xn_t16 = xnt_p.tile([P, KD, TPC], F16)  # xn^T fp16 (experts)
            h_t = ht_p.tile([P, KH, TPC], F16)      # h^T   [H, tok]
            acc = acc_p.tile([P, TOK_TILES, O], F32)  # out   [tok, O]
            w_sb = route.tile([P, TOK_TILES, E], F32)  # combine weights

            # ---- LayerNorm + transpose + router + top2 weights ----
            for i in range(TOK_TILES):
                xt = ln_p.tile([P, D], F32, name="xt")
                nc.sync.dma_start(out=xt[:], in_=x_d[i * P:(i + 1) * P, :])
                s = ln_p.tile([P, 1], F32, name="s")
                nc.vector.reduce_sum(s[:], xt[:], axis=AX)
                mu = ln_p.tile([P, 1], F32, name="mu")
                nc.scalar.mul(mu[:], s[:], 1.0 / D)
                xc = xt
                nc.vector.tensor_sub(xc[:], xt[:], mu[:].to_broadcast([P, D]))
                sq = ln_p.tile([P, D], F32, name="sq")
                nc.vector.tensor_mul(sq[:], xc[:], xc[:])
                ssq = ln_p.tile([P, 1], F32, name="ssq")
                nc.vector.reduce_sum(ssq[:], sq[:], axis=AX)
                vtmp = ln_p.tile([P, 1], F32, name="vtmp")
                nc.scalar.mul(vtmp[:], ssq[:], 1.0 / D)
                std = ln_p.tile([P, 1], F32, name="std")
                nc.scalar.activation(std[:], vtmp[:],
                                     mybir.ActivationFunctionType.Sqrt,
                                     bias=eps_t[:])
                rstd = ln_p.tile([P, 1], F32, name="rstd")
                nc.vector.reciprocal(rstd[:], std[:])
                xn = sq
                nc.vector.tensor_scalar_mul(xn[:], xc[:], rstd[:])

                # transpose 128x128 blocks into xn_t
                for k in range(KD):
                    pt = ps1p.tile([P, 512], F32, name="ps1")[:, :P]
                    nc.tensor.transpose(pt[:], xn[:, k * P:(k + 1) * P],
                                        ident[:])
                    nc.vector.tensor_scalar(
                        out=xn_t[:, k, i * P:(i + 1) * P], in0=pt[:],
                        scalar1=lns_pc[:, k:k + 1], scalar2=lnb_pc[:, k:k + 1],
                        op0=mybir.AluOpType.mult, op1=mybir.AluOpType.add)
                    nc.vector.tensor_copy(
                        xn_t16[:, k, i * P:(i + 1) * P],
                        xn_t[:, k, i * P:(i + 1) * P])

                # router logits for this token tile
                pg = ps1p.tile([P, 512], F32, name="ps1")[:, :E]
                for k in range(KD):
                    nc.tensor.matmul(
                        pg[:], lhsT=xn_t[:, k, i * P:(i + 1) * P],
                        rhs=rwt_sb[:, k, :],
                        start=(k == 0), stop=(k == KD - 1))
                gt = ln_p.tile([P, E], F32, name="gt")
                nc.vector.tensor_copy(gt[:], pg[:])
                nc.sync.dma_start(out=gate_d[i * P:(i + 1) * P, :], in_=gt[:])

                # top-2 softmax -> dense combine weights, masked
                mx = ln_p.tile([P, 8], F32, name="mx")
                nc.vector.max(out=mx[:], in_=gt[:])
                df = ln_p.tile([P, E], F32, name="df")
                nc.vector.tensor_sub(df[:], gt[:],
                                     mx[:, 0:1].to_broadcast([P, E]))
                ex = ln_p.tile([P, E], F32, name="ex")
                nc.scalar.activation(ex[:], df[:], EXP)
                mge = ln_p.tile([P, E], F32, name="mge")
                nc.vector.tensor_tensor(out=mge[:], in0=gt[:],
                                        in1=mx[:, 1:2].to_broadcast([P, E]),
                                        op=mybir.AluOpType.is_ge)
                num = ln_p.tile([P, E], F32, name="num")
                nc.vector.tensor_mul(num[:], ex[:], mge[:])
                den = ln_p.tile([P, 1], F32, name="den")
                nc.vector.reduce_sum(den[:], num[:], axis=AX)
                rden = ln_p.tile([P, 1], F32, name="rden")
                nc.vector.reciprocal(rden[:], den[:])
                nc.vector.tensor_mul(rden[:], rden[:], sm_sb[:, i, :])
                nc.vector.tensor_scalar_mul(w_sb[:, i, :], num[:], rden[:])

            # ---- dense expert MLPs ----
            for e in range(E):
                w1_sb = w1_p.tile([P, KD, H], F16, name="w1sb")
                for k in range(KD):
                    wr = wraw_p.tile([P, H], F32, name="wr")
                    nc.sync.dma_start(
                        out=wr[:], in_=w1_d[e, k * P:(k + 1) * P, :])
                    nc.vector.tensor_copy(w1_sb[:, k, :], wr[:])

                # layer 1: h^T[H, tok] = gelu(w1^T xn + b1)
                for m in range(MH):
                    ps = ps1p.tile([P, 512], F32, name="ps1")
                    for k in range(KD):
                        nc.tensor.matmul(
                            ps[:], lhsT=w1_sb[:, k, m * P:(m + 1) * P],
                            rhs=xn_t16[:, k, :],
                            start=(k == 0), stop=(k == KD - 1))
                    nc.scalar.activation(h_t[:, m, :], ps[:], GELU,
                                         bias=b1t_sb[:, e, m:m + 1])

                # layer 2: out[tok, O] += w[tok,e] * gelu(h w2 + b2)
                b2row = b2r_p.tile([1, O], F32, name="b2row")
                nc.sync.dma_start(out=b2row[:], in_=b2_d[e][None, :])
                b2bc = b2_p.tile([P, O], F32, name="b2bc")
                nc.gpsimd.partition_broadcast(b2bc[:], b2row[:])
                for o in range(2):
                    pts = [ps2p.tile([P, 512], F32, name=f"ps2_{t}")
                           for t in range(TOK_TILES)]
                    for k in range(KH):
                        w2r = wraw_p.tile([P, 512], F32, name="w2r")
                        nc.sync.dma_start(
                            out=w2r[:],
                            in_=w2_d[e, k * P:(k + 1) * P,
                                     o * 512:(o + 1) * 512])
                        w2k = w2_p.tile([P, 512], F16, name="w2k")
                        nc.scalar.activation(
                            w2k[:], w2r[:],
                            mybir.ActivationFunctionType.Copy)
                        for t in range(TOK_TILES):
                            nc.tensor.matmul(
                                pts[t][:],
                                lhsT=h_t[:, k, t * P:(t + 1) * P],
                                rhs=w2k[:],
                                start=(k == 0), stop=(k == KH - 1))
                    for t in range(TOK_TILES):
                        y = ev_p.tile([P, 512], F32, name="y")
                        nc.vector.tensor_add(y[:], pts[t][:],
                                             b2bc[:, o * 512:(o + 1) * 512])
                        nc.scalar.activation(y[:], y[:], GELU)
                        osl = acc[:, t, o * 512:(o + 1) * 512]
                        if e == 0:
                            nc.vector.tensor_scalar_mul(
                                osl, y[:], w_sb[:, t, e:e + 1])
                        else:
                            nc.vector.tensor_scalar_mul(
                                y[:], y[:], w_sb[:, t, e:e + 1])
                            nc.vector.tensor_add(osl, osl, y[:])

            for t in range(TOK_TILES):
                nc.sync.dma_start(out=out_d[t * P:(t + 1) * P, :],
                                  in_=acc[:, t, :])

    nc.compile()
    return nc


_NC_CACHE = None


def _get_nc():
    global _NC_CACHE
    if _NC_CACHE is None:
        _NC_CACHE = _build_kernel()
    return _NC_CACHE


def kernel(x, sequence_mask, ln_scale, ln_bias, router_w, w1, b1, w2, b2):
    x = np.asarray(x, np.float32).reshape(NTOK, D)
    sm = np.asarray(sequence_mask, np.float32).reshape(NTOK, 1)
    lns = np.asarray(ln_scale, np.float32).reshape(1, D)
    lnb = np.asarray(ln_bias, np.float32).reshape(1, D)
    rwt = np.ascontiguousarray(np.asarray(router_w, np.float32).T)
    w1 = np.ascontiguousarray(np.asarray(w1, np.float32))
    b1 = np.ascontiguousarray(np.asarray(b1, np.float32))
    w2 = np.ascontiguousarray(np.asarray(w2, np.float32))
    b2 = np.ascontiguousarray(np.asarray(b2, np.float32))

    nc = _get_nc()
    in_maps = []
    for c in range(NCORES):
        sl = slice(c * TPC, (c + 1) * TPC)
        in_maps.append({
            "x": np.ascontiguousarray(x[sl]),
            "seqmask": np.ascontiguousarray(sm[sl]),
            "ln_scale": lns, "ln_bias": lnb, "rwT": rwt,
            "w1": w1, "b1": b1, "w2": w2, "b2": b2,
        })
    res = run_bass_kernel_spmd(nc, in_maps, core_ids=list(range(NCORES)))
    out = np.concatenate([res.results[c]["out"] for c in range(NCORES)], 0)
    gate = np.concatenate([res.results[c]["gate"] for c in range(NCORES)], 0)
    return out.reshape(T, B, O), gate


# revision 13
# speedup vs baseline: 1.2126x; 1.0274x over previous
"""Trainium2 Bass kernel for nn_MoELayer (top-2-of-8 MoE, T=1024 B=4 D=1024 H=2048).

Strategy: pure data-parallel over tokens. Each of the 8 NeuronCores takes
512 tokens and computes layernorm -> router -> top-2 softmax weights ->
dense 8-expert MLP (fp32r matmuls) -> weighted combine, locally.
No collectives. Host only reshapes/slices inputs and concatenates outputs.
"""

import sys

sys.path.insert(0, "/opt/trn_rl_repo")

import numpy as np

import concourse.bacc as bacc
import concourse.mybir as mybir
import concourse.tile as tile
from concourse.bass_utils import run_bass_kernel_spmd
from concourse.masks import make_identity

T, B, D = 1024, 4, 1024
E, TOPK = 8, 2
H = 2048
O = D
NCORES = 8
NTOK = T * B          # 4096
TPC = NTOK // NCORES  # 512 tokens per core
P = 128
TOK_TILES = TPC // P  # 4
KD = D // P           # 8  k-chunks for D contraction
KH = H // P           # 16 k-chunks for H contraction
MH = H // P           # 16 m-tiles of H
F32 = mybir.dt.float32
FR = mybir.dt.float32r
F16 = mybir.dt.float16
AX = mybir.AxisListType.X
GELU = mybir.ActivationFunctionType.Gelu
EXP = mybir.ActivationFunctionType.Exp
RSQRT = mybir.ActivationFunctionType.Rsqrt


def _build_kernel():
    nc = bacc.Bacc("TRN2", target_bir_lowering=False, debug=False,
                   num_devices=NCORES)

    x_d = nc.dram_tensor("x", [TPC, D], F32, kind="ExternalInput").ap()
    sm_d = nc.dram_tensor("seqmask", [TPC, 1], F32, kind="ExternalInput").ap()
    lns_d = nc.dram_tensor("ln_scale", [1, D], F32, kind="ExternalInput").ap()
    lnb_d = nc.dram_tensor("ln_bias", [1, D], F32, kind="ExternalInput").ap()
    rwt_d = nc.dram_tensor("rwT", [D, E], FR, kind="ExternalInput").ap()
    w1_d = nc.dram_tensor("w1", [E, D, H], F16, kind="ExternalInput").ap()
    b1_d = nc.dram_tensor("b1", [E, H], F32, kind="ExternalInput").ap()
    w2_d = nc.dram_tensor("w2", [E, H, O], F16, kind="ExternalInput").ap()
    b2_d = nc.dram_tensor("b2", [E, O], F32, kind="ExternalInput").ap()
    out_d = nc.dram_tensor("out", [TPC, O], F32, kind="ExternalOutput").ap()
    gate_d = nc.dram_tensor("gate", [TPC, E], F32, kind="ExternalOutput").ap()

    with tile.TileContext(nc) as tc:
        with (
            tc.tile_pool(name="const", bufs=1) as const,
            tc.tile_pool(name="route", bufs=1) as route,
            tc.tile_pool(name="xnt", bufs=1) as xnt_p,
            tc.tile_pool(name="ht", bufs=1) as ht_p,
            tc.tile_pool(name="accp", bufs=1) as acc_p,
            tc.tile_pool(name="w1p", bufs=1) as w1_p,
            tc.tile_pool(name="wraw", bufs=2) as wraw_p,
            tc.tile_pool(name="w2p", bufs=3) as w2_p,
            tc.tile_pool(name="ln", bufs=2) as ln_p,
            tc.tile_pool(name="ev", bufs=4) as ev_p,
            tc.tile_pool(name="b2b", bufs=2) as b2_p,
            tc.tile_pool(name="b2r", bufs=1) as b2r_p,
            tc.tile_pool(name="ps1p", bufs=4, space="PSUM") as ps1p,
            tc.tile_pool(name="ps2p", bufs=1, space="PSUM") as ps2p,
        ):
            ident = const.tile([P, P], F32)
            make_identity(nc, ident[:])
            eps_t = const.tile([P, 1], F32)
            nc.vector.memset(eps_t[:], 1e-5)

            # ln scale/bias, one column per D-chunk (per-partition in xn_T)
            lns_pc = const.tile([P, KD], F32)
            nc.sync.dma_start(
                out=lns_pc[:], in_=lns_d.rearrange("o (k p) -> p (o k)", p=P))
            lnb_pc = const.tile([P, KD], F32)
            nc.sync.dma_start(
                out=lnb_pc[:], in_=lnb_d.rearrange("o (k p) -> p (o k)", p=P))

            rwt_sb = const.tile([P, KD, E], FR)
            nc.sync.dma_start(
                out=rwt_sb[:], in_=rwt_d.rearrange("(k p) e -> p k e", p=P))
            b1t_sb = const.tile([P, E, H // P], F32)
            nc.sync.dma_start(
                out=b1t_sb[:], in_=b1_d.rearrange("e (m p) -> p e m", p=P))
            sm_sb = const.tile([P, TOK_TILES, 1], F32)
            nc.sync.dma_start(
                out=sm_sb[:], in_=sm_d.rearrange("(t p) o -> p t o", p=P))

            xn_t = xnt_p.tile([P, KD, TPC], FR)     # xn^T  [D, tok] (router)
            xn_t16 = xnt_p.tile([P, KD, TPC], F16)  # xn^T fp16 (experts)
            h_t = ht_p.tile([P, KH, TPC], F16)      # h^T   [H, tok]
            acc = acc_p.tile([P, TOK_TILES, O], F32)  # out   [tok, O]
            w_sb = route.tile([P, TOK_TILES, E], F32)  # combine weights

            # ---- LayerNorm + transpose + router + top2 weights ----
            for i in range(TOK_TILES):
                xt = ln_p.tile([P, D], F32, name="xt")
                nc.sync.dma_start(out=xt[:], in_=x_d[i * P:(i + 1) * P, :])
                s = ln_p.tile([P, 1], F32, name="s")
                nc.vector.reduce_sum(s[:], xt[:], axis=AX)
                mu = ln_p.tile([P, 1], F32, name="mu")
                nc.scalar.mul(mu[:], s[:], 1.0 / D)
                xc = xt
                nc.vector.tensor_sub(xc[:], xt[:], mu[:].to_broadcast([P, D]))
                sq = ln_p.tile([P, D], F32, name="sq")
                nc.vector.tensor_mul(sq[:], xc[:], xc[:])
                ssq = ln_p.tile([P, 1], F32, name="ssq")
                nc.vector.reduce_sum(ssq[:], sq[:], axis=AX)
                vtmp = ln_p.tile([P, 1], F32, name="vtmp")
                nc.scalar.mul(vtmp[:], ssq[:], 1.0 / D)
                std = ln_p.tile([P, 1], F32, name="std")
                nc.scalar.activation(std[:], vtmp[:],
                                     mybir.ActivationFunctionType.Sqrt,
                                     bias=eps_t[:])
                rstd = ln_p.tile([P, 1], F32, name="rstd")
                nc.vector.reciprocal(rstd[:], std[:])
                xn = sq
                nc.vector.tensor_scalar_mul(xn[:], xc[:], rstd[:])

                # transpose 128x128 blocks into xn_t
                for k in range(KD):
                    pt = ps1p.tile([P, 512], F32, name="ps1")[:, :P]
                    nc.tensor.transpose(pt[:], xn[:, k * P:(k + 1) * P],
                                        ident[:])
                    nc.vector.tensor_scalar(
                        out=xn_t[:, k, i * P:(i + 1) * P], in0=pt[:],
                        scalar1=lns_pc[:, k:k + 1], scalar2=lnb_pc[:, k:k + 1],
                        op0=mybir.AluOpType.mult, op1=mybir.AluOpType.add)
                    nc.vector.tensor_copy(
                        xn_t16[:, k, i * P:(i + 1) * P],
                        xn_t[:, k, i * P:(i + 1) * P])

                # router logits for this token tile
                pg = ps1p.tile([P, 512], F32, name="ps1")[:, :E]
                for k in range(KD):
                    nc.tensor.matmul(
                        pg[:], lhsT=xn_t[:, k, i * P:(i + 1) * P],
                        rhs=rwt_sb[:, k, :],
                        start=(k == 0), stop=(k == KD - 1))
                gt = ln_p.tile([P, E], F32, name="gt")
                nc.vector.tensor_copy(gt[:], pg[:])
                nc.sync.dma_start(out=gate_d[i * P:(i + 1) * P, :], in_=gt[:])

                # top-2 softmax -> dense combine weights, masked
                mx = ln_p.tile([P, 8], F32, name="mx")
                nc.vector.max(out=mx[:], in_=gt[:])
                df = ln_p.tile([P, E], F32, name="df")
                nc.vector.tensor_sub(df[:], gt[:],
                                     mx[:, 0:1].to_broadcast([P, E]))
                ex = ln_p.tile([P, E], F32, name="ex")
                nc.scalar.activation(ex[:], df[:], EXP)
                mge = ln_p.tile([P, E], F32, name="mge")
                nc.vector.tensor_tensor(out=mge[:], in0=gt[:],
                                        in1=mx[:, 1:2].to_broadcast([P, E]),
                                        op=mybir.AluOpType.is_ge)
                num = ln_p.tile([P, E], F32, name="num")
                nc.vector.tensor_mul(num[:], ex[:], mge[:])
                den = ln_p.tile([P, 1], F32, name="den")
                nc.vector.reduce_sum(den[:], num[:], axis=AX)
                rden = ln_p.tile([P, 1], F32, name="rden")
                nc.vector.reciprocal(rden[:], den[:])
                nc.vector.tensor_mul(rden[:], rden[:], sm_sb[:, i, :])
                nc.vector.tensor_scalar_mul(w_sb[:, i, :], num[:], rden[:])

            # ---- dense expert MLPs ----
            for e in range(E):
                w1_sb = w1_p.tile([P, KD, H], F16, name="w1sb")
                nc.sync.dma_start(
                    out=w1_sb[:],
                    in_=w1_d[e].rearrange("(k p) h -> p k h", p=P))

                # layer 1: h^T[H, tok] = gelu(w1^T xn + b1)
                for m in range(MH):
                    ps = ps1p.tile([P, 512], F32, name="ps1")
                    for k in range(KD):
                        nc.tensor.matmul(
                            ps[:], lhsT=w1_sb[:, k, m * P:(m + 1) * P],
                            rhs=xn_t16[:, k, :],
                            start=(k == 0), stop=(k == KD - 1))
                    nc.scalar.activation(h_t[:, m, :], ps[:], GELU,
                                         bias=b1t_sb[:, e, m:m + 1])

                # layer 2: out[tok, O] += w[tok,e] * gelu(h w2 + b2)
                b2row = b2r_p.tile([1, O], F32, name="b2row")
                nc.sync.dma_start(out=b2row[:], in_=b2_d[e][None, :])
                b2bc = b2_p.tile([P, O], F32, name="b2bc")
                nc.gpsimd.partition_broadcast(b2bc[:], b2row[:])
                for o in range(2):
                    pts = [ps2p.tile([P, 512], F32, name=f"ps2_{t}")
                           for t in range(TOK_TILES)]
                    for k in range(KH):
                        w2k = w2_p.tile([P, 512], F16, name="w2k")
                        nc.sync.dma_start(
                            out=w2k[:],
                            in_=w2_d[e, k * P:(k + 1) * P,
                                     o * 512:(o + 1) * 512])
                        for t in range(TOK_TILES):
                            nc.tensor.matmul(
                                pts[t][:],
                                lhsT=h_t[:, k, t * P:(t + 1) * P],
                                rhs=w2k[:],
                                start=(k == 0), stop=(k == KH - 1))
                    for t in range(TOK_TILES):
                        y = ev_p.tile([P, 512], F32, name="y")
                        nc.vector.tensor_add(y[:], pts[t][:],
                                             b2bc[:, o * 512:(o + 1) * 512])
                        nc.scalar.activation(y[:], y[:], GELU)
                        osl = acc[:, t, o * 512:(o + 1) * 512]
                        if e == 0:
                            nc.vector.tensor_scalar_mul(
                                osl, y[:], w_sb[:, t, e:e + 1])
                        else:
                            nc.vector.tensor_scalar_mul(
                                y[:], y[:], w_sb[:, t, e:e + 1])
                            nc.vector.tensor_add(osl, osl, y[:])

            for t in range(TOK_TILES):
                nc.sync.dma_start(out=out_d[t * P:(t + 1) * P, :],
                                  in_=acc[:, t, :])

    nc.compile()
    return nc


_NC_CACHE = None


def _get_nc():
    global _NC_CACHE
    if _NC_CACHE is None:
        _NC_CACHE = _build_kernel()
    return _NC_CACHE


def kernel(x, sequence_mask, ln_scale, ln_bias, router_w, w1, b1, w2, b2):
    x = np.asarray(x, np.float32).reshape(NTOK, D)
    sm = np.asarray(sequence_mask, np.float32).reshape(NTOK, 1)
    lns = np.asarray(ln_scale, np.float32).reshape(1, D)
    lnb = np.asarray(ln_bias, np.float32).reshape(1, D)
    rwt = np.ascontiguousarray(np.asarray(router_w, np.float32).T)
    w1 = np.ascontiguousarray(np.asarray(w1, np.float32))
    b1 = np.ascontiguousarray(np.asarray(b1, np.float32))
    w2 = np.ascontiguousarray(np.asarray(w2, np.float32))
    b2 = np.ascontiguousarray(np.asarray(b2, np.float32))

    nc = _get_nc()
    in_maps = []
    for c in range(NCORES):
        sl = slice(c * TPC, (c + 1) * TPC)
        in_maps.append({
            "x": np.ascontiguousarray(x[sl]),
            "seqmask": np.ascontiguousarray(sm[sl]),
            "ln_scale": lns, "ln_bias": lnb, "rwT": rwt,
            "w1": w1.astype(np.float16), "b1": b1,
            "w2": w2.astype(np.float16), "b2": b2,
        })
    res = run_bass_kernel_spmd(nc, in_maps, core_ids=list(range(NCORES)))
    out = np.concatenate([res.results[c]["out"] for c in range(NCORES)], 0)
    gate = np.concatenate([res.results[c]["gate"] for c in range(NCORES)], 0)
    return out.reshape(T, B, O), gate
